# revision 12
# baseline (speedup 1.0000x reference)
"""BiLSTM final-state kernel for 8 TRN2 NeuronCores.

Problem: B=32, T=1024, I=H=1024 bidirectional LSTM; output = final hidden
states (hf, hr) stacked to [2, 32, 1024].

Sharding (homogeneous SPMD, no collectives):
  cores 0-3: forward cell,  batch shards {0-7, 8-15, 16-23, 24-31}
  cores 4-7: reverse cell, same batch shards, x time-flipped host-side
Each core:
  Phase 1: xp.T = Wih[perm] @ x_shard.T   (bf16 matmuls, bias fused) -> DRAM
  Phase 2: 1024-step LSTM recurrence, gates computed transposed
           ([gate_rows, batch] layout) so h/c state stays transposed and
           feeds the next step's matmul directly.
"""

import numpy as np
import ml_dtypes

BF16 = ml_dtypes.bfloat16

B, T, I, H = 32, 1024, 1024, 1024
G = 4 * H           # 4096 gate rows
BL = 8              # batch per core
NCORES = 8
BLK = 32            # recurrence steps per For_i iteration
NMT = G // 128      # 32 gate-row tiles
KT = H // 128       # 8 contraction tiles


def _gate_perm():
    # New row order: mega-groups of 512 H-rows; within a mega-group the four
    # gates (i,f,g,o) appear as 4 consecutive 128-row tiles each.
    idx = []
    for mg in range(2):
        for gate in range(4):
            for j in range(4):
                base = gate * H + mg * 512 + j * 128
                idx.append(np.arange(base, base + 128))
    return np.concatenate(idx)


_PERM = _gate_perm()

_CACHE = {}


def _split_multi_waits(nc):
    """The walrus in this container accepts at most one sync-wait per
    instruction; move extra waits onto standalone EventSemaphore instrs."""
    import concourse.mybir as mybir

    n = 0
    for fn in nc.m.functions:
        for blk in fn.blocks:
            il = blk.instructions
            i = 0
            while i < len(il):
                inst = il[i]
                si = inst.sync_info
                if (si is not None and si.on_wait is not None
                        and len(si.on_wait) > 1):
                    waits = list(si.on_wait)
                    keep = waits[-1]
                    for w in waits[:-1]:
                        n += 1
                        ev = mybir.InstEventSemaphore(
                            name=f"WSPLIT-{n}", ins=[], outs=[])
                        ev.engine = inst.engine
                        ev.sync_info = mybir.SyncInfo(on_wait=[w], on_update=[])
                        il.insert(i, ev)
                        i += 1
                    si.on_wait = [keep]
                    inst.sync_info = si
                i += 1
    return n


def _build_graph():
    import concourse.bass as bass
    import concourse.mybir as mybir
    import concourse.tile as tile
    from concourse.bass import ds

    f32 = mybir.dt.float32
    bf16 = mybir.dt.bfloat16
    ACT = mybir.ActivationFunctionType

    nc = bass.Bass()

    xT2 = nc.declare_dram_parameter("xT2", [I, T * BL], bf16, isOutput=False)
    wihT = nc.declare_dram_parameter("wihT", [H, G], bf16, isOutput=False)
    whhT = nc.declare_dram_parameter("whhT", [H, G], bf16, isOutput=False)
    biasc = nc.declare_dram_parameter("biasc", [128, NMT], f32, isOutput=False)
    h0T = nc.declare_dram_parameter("h0T", [128, 64], bf16, isOutput=False)
    c0T = nc.declare_dram_parameter("c0T", [128, 64], f32, isOutput=False)
    out = nc.declare_dram_parameter("out", [128, 64], bf16, isOutput=True)

    # xp.T in DRAM: [gate_row, bt-index] bf16 (t-major: bt = t*8 + b)
    xp2T = nc.dram_tensor("xp2T", [G, T * BL], bf16)

    NBT = T * BL          # 8192
    HALF = NBT // 2       # 4096 bt-columns per phase-1 half
    CB = BLK * BL         # 256 bt-columns per recurrence block

    with tile.TileContext(nc) as tc:
        with (
            tc.tile_pool(name="big", bufs=1) as big,
            tc.tile_pool(name="psp", bufs=1, space="PSUM") as psp,
            tc.tile_pool(name="stg", bufs=4) as stg,
            tc.tile_pool(name="xpp", bufs=2) as xpp,
            tc.tile_pool(name="tmp", bufs=4) as tmp,
        ):
            # ------------- persistent tiles -------------
            w_sb = big.tile([128, KT * G], bf16)      # Wih.T, later Whh.T
            xt_sb = big.tile([128, KT * HALF], bf16)  # x.T half
            bias_sb = big.tile([128, NMT], f32)
            hT = big.tile([128, 128], bf16)           # ping-pong 2x64
            cT = big.tile([128, 64], f32)

            nc.sync.dma_start(bias_sb[:], biasc[:])
            nc.sync.dma_start(hT[:, 0:64], h0T[:])
            nc.sync.dma_start(cT[:], c0T[:])
            for k in range(KT):
                nc.sync.dma_start(
                    w_sb[:, k * G:(k + 1) * G], wihT[k * 128:(k + 1) * 128, :]
                )

            # ------------- Phase 1: input projection -------------
            for half in range(2):
                for k in range(KT):
                    nc.sync.dma_start(
                        xt_sb[:, k * HALF:(k + 1) * HALF],
                        xT2[k * 128:(k + 1) * 128, half * HALF:(half + 1) * HALF],
                    )
                for rt in range(NMT):
                    psums = [
                        psp.tile([128, 512], f32, tag=f"ps{n}", name=f"ps{n}")
                        for n in range(8)
                    ]
                    for k in range(KT):
                        lhsT = w_sb[:, k * G + rt * 128: k * G + (rt + 1) * 128]
                        for n in range(8):
                            nc.tensor.matmul(
                                psums[n][:],
                                lhsT,
                                xt_sb[:, k * HALF + n * 512: k * HALF + (n + 1) * 512],
                                start=(k == 0),
                                stop=(k == KT - 1),
                            )
                    for n in range(8):
                        st = stg.tile([128, 512], bf16, tag="st", name="st")
                        nc.vector.tensor_scalar_add(
                            st[:], psums[n][:], bias_sb[:, rt:rt + 1]
                        )
                        nc.sync.dma_start(
                            xp2T[
                                rt * 128:(rt + 1) * 128,
                                half * HALF + n * 512: half * HALF + (n + 1) * 512,
                            ],
                            st[:],
                        )

            # ------------- Phase 2: recurrence -------------
            # reload w_sb with Whh.T (WAR handled by tile deps)
            for k in range(KT):
                nc.sync.dma_start(
                    w_sb[:, k * G:(k + 1) * G], whhT[k * 128:(k + 1) * 128, :]
                )

            with tc.For_i(0, NBT, CB) as col0:
                xp_sb = xpp.tile([128, NMT * CB], bf16, tag="xp", name="xp")
                # one 3D-AP DMA for the whole 2MB block (each dynamic-offset
                # DMA burns a bounds-check register pair; 32 would exhaust
                # them)
                nc.sync.dma_start(
                    xp_sb.rearrange("p (c w) -> p c w", w=CB)[:, :, :],
                    xp2T.rearrange("(c p) w -> p c w", p=128)[:, :, ds(col0, CB)],
                )
                xp3 = xp_sb.rearrange("p (c w) -> p c w", w=CB)

                for tl in range(BLK):
                    rb = (tl % 2) * 64
                    wb = 64 - rb
                    ps = psp.tile(
                        [128, 256], f32, tag=f"ps{tl % 2}", name="gates"
                    )
                    for mt in range(NMT):
                        for k in range(KT):
                            nc.tensor.matmul(
                                ps[:, mt * 8:(mt + 1) * 8],
                                w_sb[:, k * G + mt * 128: k * G + (mt + 1) * 128],
                                hT[:, rb + k * 8: rb + k * 8 + 8],
                                start=(k == 0),
                                stop=(k == KT - 1),
                            )
                    ps3 = ps.rearrange("p (c w) -> p c w", w=8)
                    for mg in range(2):
                        mb = mg * 16
                        tt = tmp.tile([128, 128], f32, tag="tt", name="tt")
                        tt3 = tt.rearrange("p (c w) -> p c w", w=8)
                        nc.vector.tensor_add(
                            tt3[:, :, :],
                            ps3[:, mb:mb + 16, :],
                            xp3[:, mb:mb + 16, tl * 8:(tl + 1) * 8],
                        )
                        # layout: i = cols 0:32, f = 32:64, g = 64:96, o = 96:128
                        nc.scalar.activation(tt[:, 0:64], tt[:, 0:64], ACT.Sigmoid)
                        nc.scalar.activation(tt[:, 96:128], tt[:, 96:128], ACT.Sigmoid)
                        nc.scalar.activation(tt[:, 64:96], tt[:, 64:96], ACT.Tanh)
                        cs = cT[:, mg * 32:(mg + 1) * 32]
                        t1 = tmp.tile([128, 32], f32, tag="t1", name="t1")
                        t2 = tmp.tile([128, 32], f32, tag="t2", name="t2")
                        nc.vector.tensor_mul(t1[:], tt[:, 32:64], cs)
                        nc.vector.tensor_mul(t2[:], tt[:, 0:32], tt[:, 64:96])
                        nc.vector.tensor_add(cs, t1[:], t2[:])
                        th = tmp.tile([128, 32], f32, tag="th", name="th")
                        nc.scalar.activation(th[:], cs, ACT.Tanh)
                        nc.vector.tensor_mul(
                            hT[:, wb + mg * 32: wb + (mg + 1) * 32],
                            tt[:, 96:128],
                            th[:],
                        )

            nc.sync.dma_start(out[:], hT[:, 0:64])

    _split_multi_waits(nc)
    return nc


def _prep_inputs(x, h0, c0, Wih_f, Whh_f, bih_f, bhh_f, Wih_r, Whh_r, bih_r, bhh_r):
    x = np.asarray(x, np.float32)
    h0 = np.asarray(h0, np.float32)
    c0 = np.asarray(c0, np.float32)
    perm = _PERM
    in_maps = []
    for c in range(NCORES):
        d, s = c // 4, c % 4
        bs = slice(BL * s, BL * (s + 1))
        if d == 0:
            Wih, Whh, bih, bhh = Wih_f, Whh_f, bih_f, bhh_f
        else:
            Wih, Whh, bih, bhh = Wih_r, Whh_r, bih_r, bhh_r
        Wih = np.asarray(Wih, np.float32)
        Whh = np.asarray(Whh, np.float32)
        bias = np.asarray(bih, np.float32) + np.asarray(bhh, np.float32)

        xc = x[bs]                      # [8, T, I]
        if d == 1:
            xc = xc[:, ::-1, :]
        # xT2[i, t*8+b] = xc[b, t, i]
        xT2 = np.ascontiguousarray(xc.transpose(2, 1, 0).reshape(I, T * BL))

        def st(h_full, dtype):
            # [B_local, H] -> [128, 64] with col = 8*k + b
            a = h_full[bs].T.reshape(KT, 128, BL).transpose(1, 0, 2)
            return np.ascontiguousarray(a.reshape(128, KT * BL)).astype(dtype)

        in_maps.append({
            "xT2": xT2.astype(BF16),
            "wihT": np.ascontiguousarray(Wih[perm].T).astype(BF16),
            "whhT": np.ascontiguousarray(Whh[perm].T).astype(BF16),
            "biasc": np.ascontiguousarray(bias[perm].reshape(NMT, 128).T).astype(np.float32),
            "h0T": st(h0, BF16),
            "c0T": st(c0, np.float32),
        })
    return in_maps


def _gather(results):
    hf = np.zeros((B, H), np.float32)
    hr = np.zeros((B, H), np.float32)
    for c in range(NCORES):
        d, s = c // 4, c % 4
        o = np.asarray(results[c]["out"]).astype(np.float32)  # [128, 64]
        # o[p, 8k+b] = h[b, 128k+p]
        shard = o.reshape(128, KT, BL).transpose(2, 1, 0).reshape(BL, H)
        (hf if d == 0 else hr)[BL * s: BL * (s + 1)] = shard
    return np.stack([hf, hr], axis=0)


def run_spmd(in_maps, trace=False, **kw):
    from concourse.bass_utils import run_bass_kernel_spmd

    if "nc" not in _CACHE:
        _CACHE["nc"] = _build_graph()
    return run_bass_kernel_spmd(
        _CACHE["nc"], in_maps, core_ids=list(range(NCORES)), trace=trace, **kw
    )


def kernel(**inputs) -> np.ndarray:
    in_maps = _prep_inputs(**inputs)
    res = run_spmd(in_maps, trace=False)
    return _gather(res.results)
